# revision 91
# baseline (speedup 1.0000x reference)
"""Power-STFT kernel for Trainium2 (8 NeuronCores, data-parallel over batch).

Computes, for x [32, 320000] and scalar lambd:
    x <- x - mean(x, axis=1)
    power-STFT (n_fft=1024, hop=320, periodic Hann, center reflect pad)
    out = log1p(lambd * power)   -> [32, 513, 1001] fp32

Strategy per core (4 batch samples per core), v3 — folded DFT in fp16,
fp16 output, v-halved matmul groups, pipelined head/tail:
  - Window/trig symmetry: win(1024-n) = win(n), cos sym / sin antisym about
    n=512, so the windowed DFT reduces to a length-512 contraction over
      u_t[n] = y_t[n] + y_t[1024-n],  v_t[n] = y_t[n] - y_t[1024-n]
    (n = 1..512; u[512] = 2*y[512] absorbed with half weight; n=0 has
    win(0) = 0). This HALVES the tensor-engine work vs the direct 8-chunk
    form: 4 contraction chunks of 128 per trig.
  - All matmul inputs are fp16 (full PE rate). Host ships four slab views
    of the reflect-padded signal so every fold operand is partition-
    aligned: xa/xb (forward, offsets +1/+65 so contraction slot (c,p) =
    sample 128c+p+1) and xrA/xrB (reversed, densely packed 4v+c columns:
    xp[base + 640v - 128c - p] for base 1023/1343) for even/odd frames.
    u/v are built by 8 DVE adds/subs per sample (fp16 2x mode, high
    priority, split by v-half so matmuls start off the first DMA chunks)
    into a [128, v*8+par*4+c] layout.
  - Main matmuls per (kb, h-half): ONE accumulation group per PSUM bank
    with a 2D (par, v) out AP covering both frame parities — 4 c-chunk
    matmuls, N=502. pc/ps col layout 512h+256par+v, 2 banks each, both
    double-buffered (8 banks exactly). The very first group (s0,kb0,h0)
    is par-sub-split (one start/stop pair) so streaming begins before
    xb/xrB land. Weight DMAs are kb-major and interleaved with the lead
    slab chunks.
  - Epilogue per kb in one pass over both h quadrants (packed tiles
    502h+251par+v'): cos^2 ACT Square (PSUM->fp16, kb0 adds the de-mean
    bias), sin^2 ACT (kb0/2) or DVE copy+self-mult (kb1/3, sin matmuls
    emitted first so the copy overlaps the cos stream), power add DVE,
    Ln(x+1) ACT writing a t-interleaved fp16 osb block; per-kb/piece
    output DMAs drain while later kbs compute. The last sample's kb3
    splits into 3 pieces to shorten the final serial chain; its Nyquist
    chain runs before kb0.
  - Nyquist bin (512): data-stationary matmuls (lhsT = u frame chunks,
    rhs = folded (-1)^n window column) into a [128 frames, 8 fgroup]
    PSUM tile riding the ps rotation; Square+Ln on ACT, then transposing
    gather DMAs write output row 512 (frame t = 128*fg + p).
  - Mean removal: periodic-Hann DFT of a constant is exactly [512, -256]
    at bins 0/1 (real), so demeaning == biasing bins 0/1 of the cos part.
    Two GPSIMD full reduces (per DMA chunk) -> DVE add -> GPSIMD
    partition_broadcast -> DVE multiply with a constant column; applied
    as the per-partition bias of the kb=0 cos ACT Square.
  - Output is fp16 (tolerance is ~2e-2 on max-normalized error; fp16
    rounding adds ~5e-4), halving output DMA bytes; host casts to f32.
  - sqrt(lambd) folded into the DFT matrices so power is pre-scaled.
"""

import sys

sys.path.insert(0, "/opt/trn_rl_repo")

import numpy as np

import concourse.bacc as bacc
import concourse.bass as bass
import concourse.mybir as mybir
import concourse.tile as tile
from concourse.ap import AP
from contextlib import ExitStack

N_FFT = 1024
HOP = 320
L = 320000
PAD = N_FFT // 2  # 512
LP = L + 2 * PAD  # 321024
B = 32
NCORES = 8
SPC = B // NCORES  # 4 samples per core
T = 1 + L // HOP  # 1001 frames
NV = 501  # even-frame count; odd frames use 500 + 1 junk col
QS = 2506  # fwd slab columns
QR = 4 * NV  # rev slab columns (densely packed 4v+c)
NBINS = 513

_f32 = mybir.dt.float32
_f16 = mybir.dt.float16


def _ap3(t, col_off, s1, n1, s2, n2):
    """[128, n1, n2] AP on tile t with free strides (s1, s2) from col_off."""
    base = t[:, 0:1]
    return AP(base.tensor, base.offset + col_off,
              [list(base.ap[0]), [s1, n1], [s2, n2]])


def _ap4(t, col_off, s1, n1, s2, n2, s3, n3):
    """[128, n1, n2, n3] AP on tile t with free strides (s1, s2, s3)."""
    base = t[:, 0:1]
    return AP(base.tensor, base.offset + col_off,
              [list(base.ap[0]), [s1, n1], [s2, n2], [s3, n3]])


def _build_module():
    nc = bacc.Bacc(None, target_bir_lowering=False, debug=False)

    xf_d = nc.dram_tensor("xf", [SPC, 2, 128, QS], _f16, kind="ExternalInput")
    xr_d = nc.dram_tensor("xr", [SPC, 2, 128, QR], _f16, kind="ExternalInput")
    wc_d = nc.dram_tensor("wc", [4, 128, 4, 128], _f16, kind="ExternalInput")
    ws_d = nc.dram_tensor("ws", [4, 128, 4, 128], _f16, kind="ExternalInput")
    wny_d = nc.dram_tensor("wny", [128, 4], _f16, kind="ExternalInput")
    tmpl_d = nc.dram_tensor("tmpl", [128, 1], _f32, kind="ExternalInput")
    o_d = nc.dram_tensor("o", [SPC, NBINS, T], _f16, kind="ExternalOutput")

    with tile.TileContext(nc) as tc:
        with ExitStack() as ctx:
            consts = ctx.enter_context(tc.tile_pool(name="consts", bufs=1))
            slabs = ctx.enter_context(tc.tile_pool(name="slabs", bufs=2))
            uvs = ctx.enter_context(tc.tile_pool(name="uvs", bufs=3))
            stats = ctx.enter_context(tc.tile_pool(name="stats", bufs=2))
            tmps = ctx.enter_context(tc.tile_pool(name="tmps", bufs=3))
            outs = ctx.enter_context(tc.tile_pool(name="outs", bufs=2))
            pmain = ctx.enter_context(tc.tile_pool(name="pmain", bufs=1, space="PSUM"))

            # v-range halves: (v0, nvh) — h0 sized so first-chunk DMAs
            # (cols < CSPLIT/CSPLITR) cover all its fold reads
            HALVES = ((0, 251), (251, 250))
            CSPLIT = 1280
            CSPLITR = 1024

            # weights kb-major; par0's slabs (xa/xrA) first, kb0's cos
            # slice next so the first fold+matmul group starts earliest
            wc_sb = consts.tile([128, 4, 4, 128], _f16)  # [p, kb, c, k]
            ws_sb = consts.tile([128, 4, 4, 128], _f16)

            slab_tiles = []
            for s in range(SPC):
                tiles = []
                for tg, src_d, q in (("xa", xf_d, QS), ("xb", xf_d, QS),
                                     ("xrA", xr_d, QR), ("xrB", xr_d, QR)):
                    t = slabs.tile([128, q], _f16, tag=tg, name=f"{tg}{s}")
                    tiles.append(t)
                slab_tiles.append(tiles)
                if s == 0:
                    nc.sync.dma_start(out=tiles[0][:, 0:CSPLIT],
                                      in_=xf_d[s, 0, :, 0:CSPLIT])
                    nc.sync.dma_start(out=tiles[2][:, 0:CSPLITR],
                                      in_=xr_d[s, 0, :, 0:CSPLITR])
                    nc.sync.dma_start(out=wc_sb[:, 0], in_=wc_d[0])
                    nc.sync.dma_start(out=tiles[1][:, 0:CSPLIT],
                                      in_=xf_d[s, 1, :, 0:CSPLIT])
                    nc.sync.dma_start(out=tiles[3][:, 0:CSPLITR],
                                      in_=xr_d[s, 1, :, 0:CSPLITR])
                    nc.sync.dma_start(out=ws_sb[:, 0], in_=ws_d[0])
            nc.sync.dma_start(out=slab_tiles[0][0][:, CSPLIT:QS],
                              in_=xf_d[0, 0, :, CSPLIT:QS])
            nc.sync.dma_start(out=slab_tiles[0][2][:, CSPLITR:QR],
                              in_=xr_d[0, 0, :, CSPLITR:QR])
            nc.sync.dma_start(out=slab_tiles[0][1][:, CSPLIT:QS],
                              in_=xf_d[0, 1, :, CSPLIT:QS])
            nc.sync.dma_start(out=slab_tiles[0][3][:, CSPLITR:QR],
                              in_=xr_d[0, 1, :, CSPLITR:QR])
            for kb in range(1, 4):
                nc.sync.dma_start(out=wc_sb[:, kb], in_=wc_d[kb])
                nc.sync.dma_start(out=ws_sb[:, kb], in_=ws_d[kb])
            wny_sb = consts.tile([128, 4], _f16)
            nc.sync.dma_start(out=wny_sb, in_=wny_d[:, :])
            tmplP = consts.tile([128, 1], _f32)
            nc.sync.dma_start(out=tmplP, in_=tmpl_d[:, :])

            def load_slabs(s):
                tiles = slab_tiles[s]
                for par in range(2):
                    nc.sync.dma_start(out=tiles[par][:, 0:CSPLIT],
                                      in_=xf_d[s, par, :, 0:CSPLIT])
                    nc.sync.dma_start(out=tiles[2 + par][:, 0:CSPLITR],
                                      in_=xr_d[s, par, :, 0:CSPLITR])
                for par in range(2):
                    nc.sync.dma_start(out=tiles[par][:, CSPLIT:QS],
                                      in_=xf_d[s, par, :, CSPLIT:QS])
                    nc.sync.dma_start(out=tiles[2 + par][:, CSPLITR:QR],
                                      in_=xr_d[s, par, :, CSPLITR:QR])

            for s in range(SPC):
                tiles = slab_tiles[s]
                if s != 0:
                    load_slabs(s)
                xa, xrevA, xrevB = tiles[0], tiles[2], tiles[3]
                xb = tiles[1]

                # fold: u/v [128, 8v+4par+c], even par=0 from xa/xrevA,
                # odd par=1 from xb/xrevB (odd v=500 col is finite junk);
                # rev slabs are host-gathered in forward column order so all
                # strides are positive
                u = uvs.tile([128, 8 * NV], _f16, tag="u", name=f"u{s}")
                v = uvs.tile([128, 8 * NV], _f16, tag="v", name=f"v{s}")
                with tc.high_priority():
                    # folds gate the next sample's matmuls — schedule them
                    # ahead of epilogue work contending for DVE
                    for v0, nvh in HALVES:
                        for par, fwd, rev, off_f in ((0, xa, xrevA, 0),
                                                     (1, xb, xrevB, 2)):
                            in0 = _ap3(fwd, off_f + 5 * v0, 5, nvh, 1, 4)
                            in1 = _ap3(rev, 4 * v0, 4, nvh, 1, 4)
                            uo = _ap3(u, 4 * par + 8 * v0, 8, nvh, 1, 4)
                            vo = _ap3(v, 4 * par + 8 * v0, 8, nvh, 1, 4)
                            nc.vector.tensor_add(out=uo, in0=in0, in1=in1)
                            nc.vector.tensor_sub(out=vo, in0=in0, in1=in1)

                # mean chain: sum(xa[:, 4:2504]) == sum(x) up to two edge
                # samples (~1e-5 relative on mu — far below tolerance);
                # S = sS2[0]+sS2[1] broadcast to all partitions by a
                # stride-0 DMA, then bias = tmplcol * S on DVE (tmpl
                # carries /L and sqrt(lambd) scaling; rows 0/1 nonzero)
                sS2 = stats.tile([1, 2], _f32, tag="sS", name=f"sS{s}")
                Sb = stats.tile([1, 1], _f32, tag="Sb", name=f"Sb{s}")
                Sbrd = stats.tile([128, 1], _f32, tag="Sbrd", name=f"Sbrd{s}")
                bias_sb = stats.tile([128, 1], _f32, tag="bias", name=f"bias{s}")
                with tc.high_priority():
                    # early: the reduces are the slabs' last readers, and
                    # Pool's strict-FIFO SEQ must not stall on a late DVE add
                    nc.gpsimd.reduce_sum(out=sS2[:, 0:1], in_=xa[:, 4:CSPLIT],
                                         axis=mybir.AxisListType.XYZWC)
                    nc.gpsimd.reduce_sum(out=sS2[:, 1:2],
                                         in_=xa[:, CSPLIT:2504],
                                         axis=mybir.AxisListType.XYZWC)
                    nc.vector.tensor_add(out=Sb, in0=sS2[:, 0:1],
                                         in1=sS2[:, 1:2])
                    nc.gpsimd.partition_broadcast(Sbrd[:, :], Sb[:, :])
                    nc.vector.tensor_mul(out=bias_sb, in0=tmplP[:, :],
                                         in1=Sbrd)



                def emit_ny(s=s, u=u):
                    # Nyquist bin: data-stationary chains, frames on out
                    # partitions; pny rides the ps rotation
                    misc = pmain.tile([128, 1024], _f32, tag="ps", bufs=2,
                                      name=f"misc{s}")
                    pny = misc[:, 0:8]
                    nc.vector.memset(misc[:, 7:8], 0.0)  # fg7 pad rows
                    ubase = u[:, 0:1]
                    for fg in range(8):
                        nv = 64 if fg < 7 else 53
                        m = 2 * nv
                        for c in range(4):
                            lhsT = AP(ubase.tensor,
                                      ubase.offset + 8 * 64 * fg + c,
                                      [list(ubase.ap[0]), [8, nv], [4, 2]])
                            nc.tensor.matmul(pny[0:m, fg:fg + 1], lhsT=lhsT,
                                             rhs=wny_sb[:, c:c + 1],
                                             start=(c == 0), stop=(c == 3))
                    syT = stats.tile([128, 8], _f16, tag="syT", name=f"syT{s}")
                    nc.scalar.activation(
                        out=syT, in_=pny,
                        func=mybir.ActivationFunctionType.Square)
                    nyrow = stats.tile([128, 8], _f16, tag="nyrow",
                                       name=f"ny{s}")
                    nc.scalar.activation(
                        out=nyrow, in_=syT,
                        func=mybir.ActivationFunctionType.Ln, bias=1.0)
                    # frame t = 128*fg + p: transposing gather DMAs
                    nc.sync.dma_start(
                        out=o_d[s, 512:513, 0:896].rearrange(
                            "one (fg p) -> p (one fg)", fg=7),
                        in_=nyrow[:, 0:7],
                    )
                    nc.sync.dma_start(
                        out=o_d[s, 512:513, 896:1001].rearrange(
                            "one p -> p one"),
                        in_=nyrow[0:105, 7:8])

                # pc/ps col layout: 512*h + 256*par + v' (one accumulation
                # group per (kb, h) = per bank, 2D (par, v') out AP);
                # t1/t2/cp/pw are quadrant-major packed: 502*h + 251*par + v'
                osb = outs.tile([128, 4 * 1002 + 2], _f16, tag="osb", name=f"osb{s}")
                if s == SPC - 1:
                    emit_ny()  # off the kernel tail: before kb0 (u is ready)
                for kb in range(4):
                    pc = pmain.tile([128, 1024], _f32, tag="pc", bufs=2, name=f"pc{s}_{kb}")
                    ps_ = pmain.tile([128, 1024], _f32, tag="ps", bufs=2, name=f"ps{s}_{kb}")
                    tail_kb = (s == SPC - 1 and kb == 3)
                    sin_dve = kb in (1, 3) or (s == SPC - 1 and kb == 2)
                    for h, (v0, nvh) in enumerate(HALVES):
                        # DVE-sin kbs: sin first so the DVE copy path starts
                        # while the cos matmuls still stream
                        trigs = (((ps_, ws_sb, v), (pc, wc_sb, u)) if sin_dve
                                 else ((pc, wc_sb, u), (ps_, ws_sb, v)))
                        # very first group: par-split subs (same bank, one
                        # start/stop pair) so par0 streams before xb/xrB land
                        if s == 0 and kb == 0 and h == 0:
                            par_subs = ((0, 1), (1, 1))
                        else:
                            par_subs = ((0, 2),)
                        for dst, wmat, src in trigs:
                            for si, (p0, np_) in enumerate(par_subs):
                                for c in range(4):
                                    rhs = _ap3(src, 8 * v0 + 4 * p0 + c,
                                               4, np_, 8, nvh)
                                    nc.tensor.matmul(
                                        _ap3(dst, 512 * h + 256 * p0,
                                             256, np_, 1, nvh),
                                        lhsT=wmat[:, kb, c, :],
                                        rhs=rhs,
                                        start=(c == 0 and si == 0),
                                        stop=(c == 3 and
                                              si == len(par_subs) - 1))
                    # epilogue pieces (h, nh, v0p, nvp): single pass
                    # normally; split near the kernel tail (last sample's
                    # kb2/kb3) to shorten the final serial chain
                    t1 = tmps.tile([128, 1004], _f16, tag="t1", name=f"t1{s}_{kb}")
                    t2 = tmps.tile([128, 1004], _f16, tag="t2", name=f"t2{s}_{kb}")
                    pw = tmps.tile([128, 1004], _f16, tag="pw", name=f"pw{s}_{kb}")
                    if tail_kb:
                        pieces = ((0, 1, 0, 251), (1, 1, 0, 180),
                                  (1, 1, 180, 71))
                    elif s == SPC - 1 and kb == 2:
                        pieces = ((0, 1, 0, 251), (1, 1, 0, 251))
                    else:
                        pieces = ((0, 2, 0, 251),)
                    import contextlib
                    prio = (tc.high_priority() if (s == SPC - 1 and kb >= 2)
                            else contextlib.nullcontext())
                    with prio:
                     for h_, nh, v0p, nvp in pieces:
                        co_p = 512 * h_ + v0p   # psum col offset
                        co_t = 502 * h_ + v0p   # packed-tile col offset
                        pcv = _ap4(pc, co_p, 512, nh, 256, 2, 1, nvp)
                        psv = _ap4(ps_, co_p, 512, nh, 256, 2, 1, nvp)
                        t1v = _ap4(t1, co_t, 502, nh, 251, 2, 1, nvp)
                        t2v = _ap4(t2, co_t, 502, nh, 251, 2, 1, nvp)
                        nc.scalar.activation(
                            out=t1v, in_=pcv,
                            func=mybir.ActivationFunctionType.Square,
                            bias=(bias_sb[:, 0:1] if kb == 0 else 0.0))
                        if not sin_dve:  # ACT square
                            nc.scalar.activation(
                                out=t2v, in_=psv,
                                func=mybir.ActivationFunctionType.Square)
                        else:  # DVE: copy PSUM->fp16, then 2x-mode self-mult
                            cp = tmps.tile([128, 1004], _f16, tag="cp",
                                           name=f"cp{s}_{kb}")
                            cpv = _ap4(cp, co_t, 502, nh, 251, 2, 1, nvp)
                            nc.vector.tensor_copy(out=cpv, in_=psv)
                            nc.vector.tensor_mul(
                                out=t2v, in0=cpv,
                                in1=_ap4(cp, co_t, 502, nh, 251, 2, 1, nvp))
                        nc.vector.tensor_add(out=_ap4(pw, co_t, 502, nh,
                                                      251, 2, 1, nvp),
                                             in0=t1v, in1=t2v)
                        # ln1p -> t-interleaved f16 out (t = 502h + 2v' + par)
                        obase = osb[:, 0:1]
                        oap = AP(obase.tensor,
                                 obase.offset + 1002 * kb + 502 * h_ + 2 * v0p,
                                 [list(obase.ap[0]), [502, nh], [1, 2],
                                  [2, nvp]])
                        pwv = _ap4(pw, co_t, 502, nh, 251, 2, 1, nvp)
                        nc.scalar.activation(
                            out=oap, in_=pwv,
                            func=mybir.ActivationFunctionType.Ln, bias=1.0)
                        # drain these bins/frames while the rest computes;
                        # tail pieces issue from the ACT queue (config runs
                        # right after the Ln on the same SEQ — no
                        # cross-engine sem hop, ACT has no compute left)
                        tl = 502 * h_ + 2 * v0p
                        tr = min(tl + 2 * nvp, T) if nh == 1 else T
                        nc.sync.dma_start(
                            out=o_d[s, 128 * kb:128 * kb + 128, tl:tr],
                            in_=osb[:, 1002 * kb + tl:1002 * kb + tr],
                        )

                if s != SPC - 1:
                    emit_ny()



    nc.compile()
    return nc


def _host_prepare(x, lambd):
    """Build per-core slab inputs + folded DFT matrices (fp16)."""
    x = np.ascontiguousarray(x, dtype=np.float32)
    lam = float(np.asarray(lambd, dtype=np.float32))
    sq = np.sqrt(abs(lam)) if lam != 0 else 1.0

    xp = np.concatenate(
        [x[:, PAD:0:-1], x, x[:, L - 2: L - 2 - PAD: -1]], axis=1
    )  # [B, LP]
    nq = 128 * QS  # 320768 <= LP

    def slab(src, off):
        return np.ascontiguousarray(
            src[:, off:off + nq].reshape(B, QS, 128).transpose(0, 2, 1)
        ).astype(np.float16)

    xa = slab(xp, 1)   # xp[128q+p+1]
    xb = slab(xp, 65)  # xp[128q+p+65]

    # reversed-operand slabs, gathered in forward column order and packed
    # densely: xrev*[p, 4v+c] = xp[base + 640v - 128c - p]
    vv = np.arange(NV)
    cc = np.arange(4)
    pp = np.arange(128)
    idx = (640 * vv[:, None, None] - 128 * cc[None, :, None]
           - pp[None, None, :])  # [NV, 4, 128]

    def revslab(base):
        iz = np.clip(base + idx, 0, LP - 1)
        vals = xp[:, iz]  # [B, NV, 4, 128]
        return np.ascontiguousarray(
            vals.transpose(0, 3, 1, 2).reshape(B, 128, QR)).astype(np.float16)

    xrevA = revslab(1023)  # xp[640v + 1023 - 128c - p]
    xrevB = revslab(1343)  # xp[640v + 1343 - 128c - p]
    xf = np.ascontiguousarray(np.stack([xa, xb], axis=1))
    xr = np.ascontiguousarray(np.stack([xrevA, xrevB], axis=1))

    n = np.arange(1, 513, dtype=np.float64)  # contraction slots 1..512
    win = 0.5 * (1.0 - np.cos(2.0 * np.pi * n / N_FFT))
    k = np.arange(512, dtype=np.float64)
    ang = 2.0 * np.pi * np.outer(n, k) / N_FFT
    wc64 = sq * win[:, None] * np.cos(ang)
    ws64 = sq * win[:, None] * np.sin(ang)
    wc64[511, :] *= 0.5  # u[512] = 2*y[512]
    ws64[511, :] = 0.0
    wny64 = sq * win * np.cos(np.pi * n)
    wny64[511] = 0.5 * sq

    def to_pck(w):  # [512, 512k] -> [4kb, 128p, 4c, 128k], slot n=128c+p+1
        return np.ascontiguousarray(
            w.reshape(4, 128, 4, 128).transpose(2, 1, 0, 3)).astype(np.float16)

    wc = to_pck(wc64)
    ws = to_pck(ws64)
    wny = np.ascontiguousarray(
        wny64.reshape(4, 128).transpose(1, 0)).astype(np.float16)
    tmpl = np.zeros((128, 1), dtype=np.float32)
    tmpl[0, 0] = -512.0 * sq / L
    tmpl[1, 0] = 256.0 * sq / L
    return xf, xr, wc, ws, wny, tmpl


def _in_maps(xf, xr, wc, ws, wny, tmpl):
    maps = []
    for c in range(NCORES):
        sl = slice(c * SPC, (c + 1) * SPC)
        maps.append({
            "xf": np.ascontiguousarray(xf[sl]),
            "xr": np.ascontiguousarray(xr[sl]),
            "wc": wc, "ws": ws, "wny": wny, "tmpl": tmpl,
        })
    return maps


def kernel(x, lambd):
    from concourse.bass_utils import run_bass_kernel_spmd

    prep = _host_prepare(x, lambd)
    nc = _build_module()
    res = run_bass_kernel_spmd(nc, _in_maps(*prep), core_ids=list(range(NCORES)))
    out = np.concatenate([res.results[c]["o"] for c in range(NCORES)], axis=0)
    return out.astype(np.float32)


if __name__ == "__main__":
    rng = np.random.default_rng(0)
    x = rng.standard_normal((B, L), dtype=np.float32)
    out = kernel(x, np.float32(5.0))
    print(out.shape, out.dtype, out[0, :3, :3])

